# revision 7
# baseline (speedup 1.0000x reference)
"""CrossVarianceAttention Trainium2 kernel.

Sharding: data-parallel over batch B=8, one batch element per NeuronCore
(8 cores). Each core computes the full two-branch cross-attention for its
batch element; outputs are gathered (and transposed) on host.

Device layout notes (per core, one batch element):
  - activations [1024, 512] are transposed on-device to [512, 1024]
    ("T layout": feature on partitions) because every matmul contracts
    over features.
  - attention runs per (branch, head) in [k, q] layout (k on partitions)
    so that att @ V needs no transpose; per-q statistics (mean/var of the
    variance-weighting and the softmax denominator) are computed with
    tensor-engine ones/B-matrix reductions, then broadcast across
    partitions with gpsimd partition_broadcast.
  - final out_proj produces out^T [512, 1024]; host transposes back.
"""

import os
import sys
from contextlib import ExitStack

import numpy as np

for _p in ("/opt/trn_rl_repo", "/root/.axon_site/_ro/trn_rl_repo"):
    if os.path.isdir(_p) and _p not in sys.path:
        sys.path.insert(0, _p)

import concourse.bass as bass
import concourse.bacc as bacc
import concourse.mybir as mybir
from concourse import tile
from concourse.bass_utils import run_bass_kernel_spmd
from concourse.masks import make_identity

F32 = mybir.dt.float32
F16 = mybir.dt.float16
BF16 = mybir.dt.bfloat16
AX = mybir.AxisListType
OP = mybir.AluOpType
AF = mybir.ActivationFunctionType

B, N, D = 8, 1024, 512
H, DK = 8, 64
NT = N // 128          # 8 n/k tiles of 128
DB = D // 128          # 4 feature blocks of 128
CH = N // 512          # 2 free-dim chunks of 512 (fp32 matmul N limit)
SCALE = float(np.sqrt(DK))
LN_EPS = 1e-5

W_NAMES = ["q_vis", "k_vis", "v_vis", "q_ir", "k_ir", "v_ir", "out_vis", "out_ir"]


def _emit(ctx: ExitStack, tc: "tile.TileContext", io: dict):
    nc = tc.nc

    const_pool = ctx.enter_context(tc.tile_pool(name="const", bufs=1))
    ident = const_pool.tile([128, 128], F32)
    make_identity(nc, ident[:])
    ones_f16 = const_pool.tile([128, 1], F16)
    nc.vector.memset(ones_f16[:], 1.0)
    ones_bf = const_pool.tile([128, 1], BF16)
    nc.vector.memset(ones_bf[:], 1.0)
    c_eps = const_pool.tile([128, 1], F32)
    nc.vector.memset(c_eps[:], LN_EPS)
    c_half = const_pool.tile([128, 1], F32)
    nc.vector.memset(c_half[:], 0.5)

    # --- load per-feature vectors as [128, DB] columns ---
    def load_cols(name):
        t = const_pool.tile([128, DB], F32, tag=f"col_{name}", name=f"col_{name}")
        nc.sync.dma_start(t[:], io[name][:].rearrange("(a p) -> p a", p=128))
        return t

    cols = {}
    for nm in ["ln1_g", "ln1_b", "ln2_g", "ln2_b"]:
        cols[nm] = load_cols(nm)
    for nm in W_NAMES:
        cols["b_" + nm] = load_cols("b_" + nm)

    # persistent projection outputs
    projT_pool = ctx.enter_context(tc.tile_pool(name="projT", bufs=1))
    QT = {}   # [128, DB, N] f16 : Q^T/SCALE per branch (branch -> tile)
    KT = {}   # [128, DB, N] f16 : K^T per branch
    Vn = {}   # [128, NT, D] bf16: V natural per branch
    for br in ("vis", "ir"):
        QT[br] = projT_pool.tile([128, DB, N], F16, tag=f"QT_{br}", name=f"QT_{br}")
        KT[br] = projT_pool.tile([128, DB, N], F16, tag=f"KT_{br}", name=f"KT_{br}")
        Vn[br] = projT_pool.tile([128, NT, D], BF16, tag=f"V_{br}", name=f"V_{br}")

    ot_pool = ctx.enter_context(tc.tile_pool(name="ot", bufs=1))

    stats_pool = ctx.enter_context(tc.tile_pool(name="stats", bufs=1))
    mu16 = {br: stats_pool.tile([H, N], F16, tag=f"mu16_{br}", name=f"mu16_{br}") for br in ("vis", "ir")}

    # =================== Stage A: LN + transpose inputs ===================
    with ExitStack() as sctx:
        inT_pool = sctx.enter_context(tc.tile_pool(name="inT", bufs=1))
        xT = {}
        for nm in ("lnT_vis", "lnT_ir", "fusT_rgb", "fusT_ir"):
            xT[nm] = inT_pool.tile([128, DB, N], F32, tag=nm, name=nm)

        a_pool = sctx.enter_context(tc.tile_pool(name="stA", bufs=4))
        st_pool = sctx.enter_context(tc.tile_pool(name="stA_stats", bufs=8))
        pT_pool = sctx.enter_context(
            tc.tile_pool(name="stA_psum", bufs=2, space="PSUM")
        )

        def ln_transpose(src_ap, gname, bname, dst):
            g, b = cols[gname], cols[bname]
            for nt in range(NT):
                x = a_pool.tile([128, D], F32, tag="x_in")
                nc.sync.dma_start(x[:], src_ap[nt * 128:(nt + 1) * 128, :])
                ssum = st_pool.tile([128, 1], F32, tag="ssum")
                nc.vector.tensor_reduce(ssum[:], x[:], AX.X, OP.add)
                sq = a_pool.tile([128, D], F32, tag="sq_scratch")
                sqsum = st_pool.tile([128, 1], F32, tag="sqsum")
                nc.scalar.activation(sq[:], x[:], AF.Square, accum_out=sqsum[:])
                mu = st_pool.tile([128, 1], F32, tag="mu")
                nc.vector.tensor_scalar_mul(mu[:], ssum[:], 1.0 / D)
                ex2 = st_pool.tile([128, 1], F32, tag="ex2")
                nc.vector.tensor_scalar_mul(ex2[:], sqsum[:], 1.0 / D)
                mu2 = st_pool.tile([128, 1], F32, tag="mu2")
                nc.vector.tensor_mul(mu2[:], mu[:], mu[:])
                var = st_pool.tile([128, 1], F32, tag="var")
                nc.vector.tensor_sub(var[:], ex2[:], mu2[:])
                std = st_pool.tile([128, 1], F32, tag="std")
                nc.scalar.activation(std[:], var[:], AF.Sqrt, bias=c_eps[:])
                rstd = st_pool.tile([128, 1], F32, tag="rstd")
                nc.vector.reciprocal(rstd[:], std[:])
                xh = a_pool.tile([128, D], F32, tag="xhat")
                nc.vector.tensor_scalar(
                    xh[:], x[:], mu[:], rstd[:], OP.subtract, OP.mult
                )
                for kb in range(DB):
                    ps = pT_pool.tile([128, 128], F32, tag="pT")
                    nc.tensor.transpose(
                        ps[:], xh[:, kb * 128:(kb + 1) * 128], ident[:]
                    )
                    nc.vector.tensor_scalar(
                        dst[:, kb, nt * 128:(nt + 1) * 128],
                        ps[:],
                        g[:, kb:kb + 1],
                        b[:, kb:kb + 1],
                        OP.mult,
                        OP.add,
                    )

        def plain_transpose(src_ap, dst):
            for nt in range(NT):
                x = a_pool.tile([128, D], F32, tag="x_in")
                nc.sync.dma_start(x[:], src_ap[nt * 128:(nt + 1) * 128, :])
                for kb in range(DB):
                    ps = pT_pool.tile([128, 128], F32, tag="pT")
                    nc.tensor.transpose(
                        ps[:], x[:, kb * 128:(kb + 1) * 128], ident[:]
                    )
                    nc.scalar.copy(dst[:, kb, nt * 128:(nt + 1) * 128], ps[:])

        ln_transpose(io["rgb_fea"][:], "ln1_g", "ln1_b", xT["lnT_vis"])
        ln_transpose(io["ir_fea"][:], "ln2_g", "ln2_b", xT["lnT_ir"])
        plain_transpose(io["rgb_fused"][:], xT["fusT_rgb"])
        plain_transpose(io["ir_fused"][:], xT["fusT_ir"])

        # =================== Stage B: projections ===================
        w_pool = sctx.enter_context(tc.tile_pool(name="wts", bufs=2))
        pj_pool = sctx.enter_context(
            tc.tile_pool(name="stB_psum", bufs=4, space="PSUM")
        )

        def load_w(name):
            w = w_pool.tile([128, DB, D], F32, tag="W")
            nc.sync.dma_start(
                w[:], io["W_" + name][:].rearrange("(a p) o -> p a o", p=128)
            )
            return w

        def proj_T(xt, wname, dst, scale=None):
            w = load_w(wname)
            bc = cols["b_" + wname]
            for m in range(DB):
                for c in range(CH):
                    ps = pj_pool.tile([128, 512], F32, tag="pj")
                    for kb in range(DB):
                        nc.tensor.matmul(
                            ps[:],
                            w[:, kb, m * 128:(m + 1) * 128],
                            xt[:, kb, c * 512:(c + 1) * 512],
                            start=(kb == 0),
                            stop=(kb == DB - 1),
                        )
                    if scale is None:
                        nc.vector.tensor_scalar_add(
                            dst[:, m, c * 512:(c + 1) * 512], ps[:], bc[:, m:m + 1]
                        )
                    else:
                        nc.vector.tensor_scalar(
                            dst[:, m, c * 512:(c + 1) * 512],
                            ps[:],
                            bc[:, m:m + 1],
                            float(scale),
                            OP.add,
                            OP.mult,
                        )

        def proj_N(xt, wname, dst):
            # natural-layout projection (for V), bias deferred to host-side
            # identity:   sum_k P[k,q] = 1  =>  bias handled via +b after
            # normalization (added on device in out-proj stage via W^T b).
            w = load_w(wname)
            for nt in range(NT):
                ps = pj_pool.tile([128, 512], F32, tag="pj")
                for kb in range(DB):
                    nc.tensor.matmul(
                        ps[:],
                        xt[:, kb, nt * 128:(nt + 1) * 128],
                        w[:, kb, :],
                        start=(kb == 0),
                        stop=(kb == DB - 1),
                    )
                nc.vector.tensor_copy(dst[:, nt, :], ps[:])

        # branch "vis": Q from ir_fused (W_q_ir), K/V from LN(rgb_fea)
        proj_T(xT["fusT_ir"], "q_ir", QT["vis"], scale=1.0 / SCALE)
        proj_T(xT["lnT_vis"], "k_vis", KT["vis"])
        proj_N(xT["lnT_vis"], "v_vis", Vn["vis"])
        # branch "ir": Q from rgb_fused (W_q_vis), K/V from LN(ir_fea)
        proj_T(xT["fusT_rgb"], "q_vis", QT["ir"], scale=1.0 / SCALE)
        proj_T(xT["lnT_ir"], "k_ir", KT["ir"])
        proj_N(xT["lnT_ir"], "v_ir", Vn["ir"])

        # ---- Stage B2: column means of scores via B-matrix trick ----
        # mu[h, q] = (sum_k e[k, q]) / N = (ksum_head . Q^T_head)[q] / N
        b2_pool = sctx.enter_context(tc.tile_pool(name="stB2", bufs=1))
        mu_ps_pool = sctx.enter_context(
            tc.tile_pool(name="stB2_psum", bufs=1, space="PSUM")
        )
        for br in ("vis", "ir"):
            ks = b2_pool.tile([128, DB], F32, tag="ksum")
            for kb in range(DB):
                nc.vector.tensor_reduce(
                    ks[:, kb:kb + 1], KT[br][:, kb, :], AX.X, OP.add
                )
            bmat = b2_pool.tile([128, DB, H], F16, tag="bmat")
            nc.vector.memset(bmat[:], 0.0)
            for h in range(H):
                kb_h, base = h // 2, (h % 2) * 64
                nc.vector.tensor_copy(
                    bmat[base:base + 64, kb_h, h:h + 1],
                    ks[base:base + 64, kb_h:kb_h + 1],
                )
            mps = mu_ps_pool.tile([H, N], F32, tag="mu_ps")
            for c in range(CH):
                for kb in range(DB):
                    nc.tensor.matmul(
                        mps[:, c * 512:(c + 1) * 512],
                        bmat[:, kb, :],
                        QT[br][:, kb, c * 512:(c + 1) * 512],
                        start=(kb == 0),
                        stop=(kb == DB - 1),
                    )
            nc.vector.tensor_scalar_mul(mu16[br][:], mps[:], 1.0 / N)

    # =================== Stage C: attention per (branch, head) ============
    OT = {br: ot_pool.tile([128, DB, N], F32, tag=f"OT_{br}", name=f"OT_{br}") for br in ("vis", "ir")}
    cctx = ctx.enter_context(ExitStack())
    c_pool = cctx.enter_context(tc.tile_pool(name="stC", bufs=1))
    c2_pool = cctx.enter_context(tc.tile_pool(name="stC_e", bufs=1))
    row_pool = cctx.enter_context(tc.tile_pool(name="stC_rows", bufs=2))
    eps_pool = cctx.enter_context(tc.tile_pool(name="e_psum", bufs=2, space="PSUM"))
    aux_pool = cctx.enter_context(tc.tile_pool(name="aux_psum", bufs=1, space="PSUM"))
    o_pool = cctx.enter_context(tc.tile_pool(name="o_psum", bufs=1, space="PSUM"))

    for br in ("vis", "ir"):
        for h in range(H):
            kb_h, base = h // 2, (h % 2) * 64
            kt_h = KT[br][base:base + 64, kb_h, :]
            qt_h = QT[br][base:base + 64, kb_h, :]

            # per-q mean broadcast to all partitions (fp16)
            bmu = c_pool.tile([128, N], F16, tag="bmu")
            mu_row = row_pool.tile([1, N], F16, tag="mu_row")
            nc.sync.dma_start(mu_row[:], mu16[br][h:h + 1, :])
            nc.gpsimd.partition_broadcast(bmu[:], mu_row[:])

            # scores e = K^T.T @ (Q^T/SCALE)  -> [k, q]
            e16 = c2_pool.tile([128, NT, N], F16, tag="e16")
            for kt in range(NT):
                eps = eps_pool.tile([128, N], F32, tag="e_ps")
                for c in range(CH):
                    nc.tensor.matmul(
                        eps[:, c * 512:(c + 1) * 512],
                        kt_h[:, kt * 128:(kt + 1) * 128],
                        qt_h[:, c * 512:(c + 1) * 512],
                        start=True,
                        stop=True,
                    )
                nc.scalar.copy(e16[:, kt, :], eps[:])

            # t = e - mu ; c0 = t^2 ; varsum = ones-reduce(c0)
            t16 = c_pool.tile([128, NT, N], F16, tag="t16")
            for kt in range(NT):
                nc.vector.tensor_sub(t16[:, kt, :], e16[:, kt, :], bmu[:])
            c16 = c_pool.tile([128, NT, N], F16, tag="c16")
            nc.vector.tensor_mul(c16[:], t16[:], t16[:])
            vps = aux_pool.tile([1, N], F32, tag="red_ps")
            for c in range(CH):
                for kt in range(NT):
                    nc.tensor.matmul(
                        vps[:, c * 512:(c + 1) * 512],
                        ones_f16[:],
                        c16[:, kt, c * 512:(c + 1) * 512],
                        start=(kt == 0),
                        stop=(kt == NT - 1),
                    )
            # r = 1 / (2*var + 1e-6), var = varsum / N
            rr = row_pool.tile([1, N], F32, tag="rr")
            nc.vector.tensor_scalar(vps[:], vps[:], 2.0 / N, 1e-6, OP.mult, OP.add)
            nc.vector.tensor_copy(rr[:], vps[:])
            rf = row_pool.tile([1, N], F32, tag="rf")
            nc.vector.reciprocal(rf[:], rr[:])
            r16row = row_pool.tile([1, N], F16, tag="r16row")
            nc.vector.tensor_copy(r16row[:], rf[:])
            br16 = c_pool.tile([128, N], F16, tag="br16")
            nc.gpsimd.partition_broadcast(br16[:], r16row[:])

            # u = c0 * r ; w = sigmoid(u + 0.5) ; a = e * w ; ew = exp(a)
            u16 = c_pool.tile([128, NT, N], F16, tag="u16")
            for kt in range(NT):
                nc.vector.tensor_mul(u16[:, kt, :], c16[:, kt, :], br16[:])
            nc.scalar.activation(u16[:], u16[:], AF.Sigmoid, bias=c_half[:])
            a16 = c_pool.tile([128, NT, N], F16, tag="t16")
            nc.vector.tensor_mul(a16[:], e16[:], u16[:])
            ew = c2_pool.tile([128, NT, N], BF16, tag="e16", name="ew")
            nc.scalar.activation(ew[:], a16[:], AF.Exp)

            # denom + AV
            dps = aux_pool.tile([1, N], F32, tag="red_ps")
            for c in range(CH):
                for kt in range(NT):
                    nc.tensor.matmul(
                        dps[:, c * 512:(c + 1) * 512],
                        ones_bf[:],
                        ew[:, kt, c * 512:(c + 1) * 512],
                        start=(kt == 0),
                        stop=(kt == NT - 1),
                    )
            dd = row_pool.tile([1, N], F32, tag="rr")
            nc.vector.tensor_copy(dd[:], dps[:])
            rd = row_pool.tile([1, N], F32, tag="rf")
            nc.vector.reciprocal(rd[:], dd[:])
            brd = c_pool.tile([128, N], F32, tag="brd")
            nc.gpsimd.partition_broadcast(brd[:], rd[:])

            ops = o_pool.tile([64, N], F32, tag="o_ps")
            for c in range(CH):
                for kt in range(NT):
                    nc.tensor.matmul(
                        ops[:, c * 512:(c + 1) * 512],
                        Vn[br][:, kt, h * 64:(h + 1) * 64],
                        ew[:, kt, c * 512:(c + 1) * 512],
                        start=(kt == 0),
                        stop=(kt == NT - 1),
                    )
            # O^T slice, normalized by softmax denominator
            nc.vector.scalar_tensor_tensor(
                OT[br][base:base + 64, kb_h, :],
                ops[:],
                1.0,
                brd[:64, :],
                OP.mult,
                OP.mult,
            )

    cctx.close()

    # =================== Stage D: out-proj (transposed output) ============
    with ExitStack() as sctx:
        w_pool = sctx.enter_context(tc.tile_pool(name="wts_out", bufs=2))
        d_pool = sctx.enter_context(tc.tile_pool(name="stD", bufs=4))
        dp_pool = sctx.enter_context(
            tc.tile_pool(name="stD_psum", bufs=4, space="PSUM")
        )
        for br in ("vis", "ir"):
            wname = "out_" + br
            w = w_pool.tile([128, DB, D], F32, tag="Wout")
            nc.sync.dma_start(
                w[:], io["W_" + wname][:].rearrange("(a p) o -> p a o", p=128)
            )
            bout = cols["b_" + wname]
            bv = cols["b_v_" + br]
            # total bias = b_out + W_out^T b_v   (V-projection bias folded in)
            btot = d_pool.tile([128, DB], F32, tag="btot")
            for m in range(DB):
                wb = dp_pool.tile([128, 1], F32, tag="wb_ps")
                for kb in range(DB):
                    nc.tensor.matmul(
                        wb[:],
                        w[:, kb, m * 128:(m + 1) * 128],
                        bv[:, kb:kb + 1],
                        start=(kb == 0),
                        stop=(kb == DB - 1),
                    )
                nc.vector.tensor_add(btot[:, m:m + 1], wb[:], bout[:, m:m + 1])
            out_dram = io["out_vis_T"] if br == "vis" else io["out_ir_T"]
            for m in range(DB):
                for c in range(CH):
                    ps = dp_pool.tile([128, 512], F32, tag="op_ps")
                    for kb in range(DB):
                        nc.tensor.matmul(
                            ps[:],
                            w[:, kb, m * 128:(m + 1) * 128],
                            OT[br][:, kb, c * 512:(c + 1) * 512],
                            start=(kb == 0),
                            stop=(kb == DB - 1),
                        )
                    osb = d_pool.tile([128, 512], F32, tag="osb")
                    nc.vector.tensor_scalar_add(osb[:], ps[:], btot[:, m:m + 1])
                    nc.sync.dma_start(
                        out_dram[m * 128:(m + 1) * 128, c * 512:(c + 1) * 512],
                        osb[:],
                    )


def build_nc():
    nc = bacc.Bacc()
    io = {}
    for nm in ["rgb_fea", "ir_fea", "rgb_fused", "ir_fused"]:
        io[nm] = nc.declare_dram_parameter(nm, [N, D], F32, isOutput=False)
    for nm in W_NAMES:
        io["W_" + nm] = nc.declare_dram_parameter("W_" + nm, [D, D], F32, isOutput=False)
        io["b_" + nm] = nc.declare_dram_parameter("b_" + nm, [D], F32, isOutput=False)
    for nm in ["ln1_g", "ln1_b", "ln2_g", "ln2_b"]:
        io[nm] = nc.declare_dram_parameter(nm, [D], F32, isOutput=False)
    io["out_vis_T"] = nc.declare_dram_parameter("out_vis_T", [D, N], F32, isOutput=True)
    io["out_ir_T"] = nc.declare_dram_parameter("out_ir_T", [D, N], F32, isOutput=True)

    with tile.TileContext(nc) as tc:
        with ExitStack() as ctx:
            _emit(ctx, tc, io)
    nc.finalize()
    return nc


_NC_CACHE = None


def _get_nc():
    global _NC_CACHE
    if _NC_CACHE is None:
        _NC_CACHE = build_nc()
    return _NC_CACHE


def _in_maps(rgb_fea, ir_fea, rgb_fused, ir_fused, params):
    maps = []
    for i in range(B):
        m = {
            "rgb_fea": np.ascontiguousarray(rgb_fea[i], np.float32),
            "ir_fea": np.ascontiguousarray(ir_fea[i], np.float32),
            "rgb_fused": np.ascontiguousarray(rgb_fused[i], np.float32),
            "ir_fused": np.ascontiguousarray(ir_fused[i], np.float32),
        }
        for nm in W_NAMES:
            m["W_" + nm] = np.ascontiguousarray(params["W_" + nm], np.float32)
            m["b_" + nm] = np.ascontiguousarray(params["b_" + nm], np.float32)
        for nm in ["ln1_g", "ln1_b", "ln2_g", "ln2_b"]:
            m[nm] = np.ascontiguousarray(params[nm], np.float32)
        maps.append(m)
    return maps


def run(rgb_fea, ir_fea, rgb_fused, ir_fused, params, trace=False):
    nc = _get_nc()
    maps = _in_maps(
        np.asarray(rgb_fea), np.asarray(ir_fea),
        np.asarray(rgb_fused), np.asarray(ir_fused), params,
    )
    res = run_bass_kernel_spmd(nc, maps, list(range(B)), trace=trace)
    out_vis = np.stack([res.results[i]["out_vis_T"].T for i in range(B)])
    out_ir = np.stack([res.results[i]["out_ir_T"].T for i in range(B)])
    return (out_vis, out_ir), res


def kernel(rgb_fea, ir_fea, rgb_fused, ir_fused, params):
    (out_vis, out_ir), _ = run(rgb_fea, ir_fea, rgb_fused, ir_fused, params)
    return out_vis, out_ir


# revision 10
# speedup vs baseline: 1.2801x; 1.2801x over previous
"""CrossVarianceAttention Trainium2 kernel.

Sharding: data-parallel over batch B=8, one batch element per NeuronCore
(8 cores). Each core computes the full two-branch cross-attention for its
batch element; outputs are gathered (and transposed) on host.

Device layout notes (per core, one batch element):
  - activations [1024, 512] are transposed on-device to [512, 1024]
    ("T layout": feature on partitions) because every matmul contracts
    over features.
  - attention runs per (branch, head) in [k, q] layout (k on partitions)
    so that att @ V needs no transpose; per-q statistics (mean/var of the
    variance-weighting and the softmax denominator) are computed with
    tensor-engine ones/B-matrix reductions, then broadcast across
    partitions with gpsimd partition_broadcast.
  - final out_proj produces out^T [512, 1024]; host transposes back.
"""

import os
import sys
from contextlib import ExitStack

import numpy as np

for _p in ("/opt/trn_rl_repo", "/root/.axon_site/_ro/trn_rl_repo"):
    if os.path.isdir(_p) and _p not in sys.path:
        sys.path.insert(0, _p)

import concourse.bass as bass
import concourse.bacc as bacc
import concourse.mybir as mybir
from concourse import tile
from concourse.bass_utils import run_bass_kernel_spmd
from concourse.masks import make_identity

F32 = mybir.dt.float32
F16 = mybir.dt.float16
BF16 = mybir.dt.bfloat16
AX = mybir.AxisListType
OP = mybir.AluOpType
AF = mybir.ActivationFunctionType

B, N, D = 8, 1024, 512
H, DK = 8, 64
NT = N // 128          # 8 n/k tiles of 128
DB = D // 128          # 4 feature blocks of 128
CH = N // 512          # 2 free-dim chunks of 512 (fp32 matmul N limit)
SCALE = float(np.sqrt(DK))
LN_EPS = 1e-5

W_NAMES = ["q_vis", "k_vis", "v_vis", "q_ir", "k_ir", "v_ir", "out_vis", "out_ir"]


def _emit(ctx: ExitStack, tc: "tile.TileContext", io: dict):
    nc = tc.nc

    const_pool = ctx.enter_context(tc.tile_pool(name="const", bufs=1))
    ident = const_pool.tile([128, 128], F32)
    make_identity(nc, ident[:])
    ones_f16 = const_pool.tile([128, 1], F16)
    nc.vector.memset(ones_f16[:], 1.0)
    ones_bf = const_pool.tile([128, 1], BF16)
    nc.vector.memset(ones_bf[:], 1.0)
    c_eps = const_pool.tile([128, 1], F32)
    nc.vector.memset(c_eps[:], LN_EPS)
    c_half = const_pool.tile([128, 1], F32)
    nc.vector.memset(c_half[:], 0.5)

    # --- load per-feature vectors as [128, DB] columns ---
    def load_cols(name):
        t = const_pool.tile([128, DB], F32, tag=f"col_{name}", name=f"col_{name}")
        nc.sync.dma_start(t[:], io[name][:].rearrange("(a p) -> p a", p=128))
        return t

    cols = {}
    for nm in ["ln1_g", "ln1_b", "ln2_g", "ln2_b"]:
        cols[nm] = load_cols(nm)
    for nm in W_NAMES:
        cols["b_" + nm] = load_cols("b_" + nm)

    # persistent projection outputs
    projT_pool = ctx.enter_context(tc.tile_pool(name="projT", bufs=1))
    QT = {}   # [128, DB, N] f16 : Q^T/SCALE per branch (branch -> tile)
    KT = {}   # [128, DB, N] f16 : K^T per branch
    Vn = {}   # [128, NT, D] bf16: V natural per branch
    for br in ("vis", "ir"):
        QT[br] = projT_pool.tile([128, DB, N], F16, tag=f"QT_{br}", name=f"QT_{br}")
        KT[br] = projT_pool.tile([128, DB, N], F16, tag=f"KT_{br}", name=f"KT_{br}")
        Vn[br] = projT_pool.tile([128, NT, D], BF16, tag=f"V_{br}", name=f"V_{br}")

    ot_pool = ctx.enter_context(tc.tile_pool(name="ot", bufs=1))

    stats_pool = ctx.enter_context(tc.tile_pool(name="stats", bufs=1))
    mu16 = {br: stats_pool.tile([H, N], F16, tag=f"mu16_{br}", name=f"mu16_{br}") for br in ("vis", "ir")}

    # =================== Stage A: LN + transpose inputs ===================
    with ExitStack() as sctx:
        inT_pool = sctx.enter_context(tc.tile_pool(name="inT", bufs=1))
        xT = {}
        for nm in ("lnT_vis", "lnT_ir", "fusT_rgb", "fusT_ir"):
            xT[nm] = inT_pool.tile([128, DB, N], F32, tag=nm, name=nm)

        a_pool = sctx.enter_context(tc.tile_pool(name="stA", bufs=4))
        st_pool = sctx.enter_context(tc.tile_pool(name="stA_stats", bufs=8))
        pT_pool = sctx.enter_context(
            tc.tile_pool(name="stA_psum", bufs=2, space="PSUM")
        )

        def ln_transpose(src_ap, gname, bname, dst):
            g, b = cols[gname], cols[bname]
            for nt in range(NT):
                x = a_pool.tile([128, D], F32, tag="x_in")
                nc.sync.dma_start(x[:], src_ap[nt * 128:(nt + 1) * 128, :])
                ssum = st_pool.tile([128, 1], F32, tag="ssum")
                nc.vector.tensor_reduce(ssum[:], x[:], AX.X, OP.add)
                sq = a_pool.tile([128, D], F32, tag="sq_scratch")
                sqsum = st_pool.tile([128, 1], F32, tag="sqsum")
                nc.scalar.activation(sq[:], x[:], AF.Square, accum_out=sqsum[:])
                mu = st_pool.tile([128, 1], F32, tag="mu")
                nc.vector.tensor_scalar_mul(mu[:], ssum[:], 1.0 / D)
                ex2 = st_pool.tile([128, 1], F32, tag="ex2")
                nc.vector.tensor_scalar_mul(ex2[:], sqsum[:], 1.0 / D)
                mu2 = st_pool.tile([128, 1], F32, tag="mu2")
                nc.vector.tensor_mul(mu2[:], mu[:], mu[:])
                var = st_pool.tile([128, 1], F32, tag="var")
                nc.vector.tensor_sub(var[:], ex2[:], mu2[:])
                std = st_pool.tile([128, 1], F32, tag="std")
                nc.scalar.activation(std[:], var[:], AF.Sqrt, bias=c_eps[:])
                rstd = st_pool.tile([128, 1], F32, tag="rstd")
                nc.vector.reciprocal(rstd[:], std[:])
                xh = a_pool.tile([128, D], F32, tag="xhat")
                nc.vector.tensor_scalar(
                    xh[:], x[:], mu[:], rstd[:], OP.subtract, OP.mult
                )
                for kb in range(DB):
                    ps = pT_pool.tile([128, 128], F32, tag="pT")
                    nc.tensor.transpose(
                        ps[:], xh[:, kb * 128:(kb + 1) * 128], ident[:]
                    )
                    nc.vector.tensor_scalar(
                        dst[:, kb, nt * 128:(nt + 1) * 128],
                        ps[:],
                        g[:, kb:kb + 1],
                        b[:, kb:kb + 1],
                        OP.mult,
                        OP.add,
                    )

        def plain_transpose(src_ap, dst):
            for nt in range(NT):
                x = a_pool.tile([128, D], F32, tag="x_in")
                nc.sync.dma_start(x[:], src_ap[nt * 128:(nt + 1) * 128, :])
                for kb in range(DB):
                    ps = pT_pool.tile([128, 128], F32, tag="pT")
                    nc.tensor.transpose(
                        ps[:], x[:, kb * 128:(kb + 1) * 128], ident[:]
                    )
                    nc.scalar.copy(dst[:, kb, nt * 128:(nt + 1) * 128], ps[:])

        ln_transpose(io["rgb_fea"][:], "ln1_g", "ln1_b", xT["lnT_vis"])
        ln_transpose(io["ir_fea"][:], "ln2_g", "ln2_b", xT["lnT_ir"])
        plain_transpose(io["rgb_fused"][:], xT["fusT_rgb"])
        plain_transpose(io["ir_fused"][:], xT["fusT_ir"])

        # =================== Stage B: projections ===================
        w_pool = sctx.enter_context(tc.tile_pool(name="wts", bufs=2))
        pj_pool = sctx.enter_context(
            tc.tile_pool(name="stB_psum", bufs=4, space="PSUM")
        )

        def load_w(name):
            w = w_pool.tile([128, DB, D], F32, tag="W")
            nc.sync.dma_start(
                w[:], io["W_" + name][:].rearrange("(a p) o -> p a o", p=128)
            )
            return w

        def proj_T(xt, wname, dst, scale=None):
            w = load_w(wname)
            bc = cols["b_" + wname]
            for m in range(DB):
                for c in range(CH):
                    ps = pj_pool.tile([128, 512], F32, tag="pj")
                    for kb in range(DB):
                        nc.tensor.matmul(
                            ps[:],
                            w[:, kb, m * 128:(m + 1) * 128],
                            xt[:, kb, c * 512:(c + 1) * 512],
                            start=(kb == 0),
                            stop=(kb == DB - 1),
                        )
                    if scale is None:
                        nc.vector.tensor_scalar_add(
                            dst[:, m, c * 512:(c + 1) * 512], ps[:], bc[:, m:m + 1]
                        )
                    else:
                        nc.vector.tensor_scalar(
                            dst[:, m, c * 512:(c + 1) * 512],
                            ps[:],
                            bc[:, m:m + 1],
                            float(scale),
                            OP.add,
                            OP.mult,
                        )

        def proj_N(xt, wname, dst):
            # natural-layout projection (for V), bias deferred to host-side
            # identity:   sum_k P[k,q] = 1  =>  bias handled via +b after
            # normalization (added on device in out-proj stage via W^T b).
            w = load_w(wname)
            for nt in range(NT):
                ps = pj_pool.tile([128, 512], F32, tag="pj")
                for kb in range(DB):
                    nc.tensor.matmul(
                        ps[:],
                        xt[:, kb, nt * 128:(nt + 1) * 128],
                        w[:, kb, :],
                        start=(kb == 0),
                        stop=(kb == DB - 1),
                    )
                nc.vector.tensor_copy(dst[:, nt, :], ps[:])

        # branch "vis": Q from ir_fused (W_q_ir), K/V from LN(rgb_fea)
        proj_T(xT["fusT_ir"], "q_ir", QT["vis"], scale=1.0 / SCALE)
        proj_T(xT["lnT_vis"], "k_vis", KT["vis"])
        proj_N(xT["lnT_vis"], "v_vis", Vn["vis"])
        # branch "ir": Q from rgb_fused (W_q_vis), K/V from LN(ir_fea)
        proj_T(xT["fusT_rgb"], "q_vis", QT["ir"], scale=1.0 / SCALE)
        proj_T(xT["lnT_ir"], "k_ir", KT["ir"])
        proj_N(xT["lnT_ir"], "v_ir", Vn["ir"])

        # ---- Stage B2: column means of scores via B-matrix trick ----
        # mu[h, q] = (sum_k e[k, q]) / N = (ksum_head . Q^T_head)[q] / N
        b2_pool = sctx.enter_context(tc.tile_pool(name="stB2", bufs=1))
        mu_ps_pool = sctx.enter_context(
            tc.tile_pool(name="stB2_psum", bufs=1, space="PSUM")
        )
        for br in ("vis", "ir"):
            ks = b2_pool.tile([128, DB], F32, tag="ksum")
            for kb in range(DB):
                nc.vector.tensor_reduce(
                    ks[:, kb:kb + 1], KT[br][:, kb, :], AX.X, OP.add
                )
            bmat = b2_pool.tile([128, DB, H], F16, tag="bmat")
            nc.vector.memset(bmat[:], 0.0)
            for h in range(H):
                kb_h, base = h // 2, (h % 2) * 64
                nc.vector.tensor_copy(
                    bmat[base:base + 64, kb_h, h:h + 1],
                    ks[base:base + 64, kb_h:kb_h + 1],
                )
            mps = mu_ps_pool.tile([H, N], F32, tag="mu_ps")
            for c in range(CH):
                for kb in range(DB):
                    nc.tensor.matmul(
                        mps[:, c * 512:(c + 1) * 512],
                        bmat[:, kb, :],
                        QT[br][:, kb, c * 512:(c + 1) * 512],
                        start=(kb == 0),
                        stop=(kb == DB - 1),
                    )
            nc.vector.tensor_scalar_mul(mu16[br][:], mps[:], 1.0 / N)

    # =================== Stage C: attention per (branch, head) ============
    OT = {br: ot_pool.tile([128, DB, N], F16, tag=f"OT_{br}", name=f"OT_{br}") for br in ("vis", "ir")}
    cctx = ctx.enter_context(ExitStack())
    c_pool = cctx.enter_context(tc.tile_pool(name="stC", bufs=2))
    c2_pool = cctx.enter_context(tc.tile_pool(name="stC_e", bufs=3))
    bmu_pool = cctx.enter_context(tc.tile_pool(name="stC_bmu", bufs=4))
    row_pool = cctx.enter_context(tc.tile_pool(name="stC_rows", bufs=2))
    eps_pool = cctx.enter_context(tc.tile_pool(name="e_psum", bufs=4, space="PSUM"))
    aux_pool = cctx.enter_context(tc.tile_pool(name="aux_psum", bufs=2, space="PSUM"))
    o_pool = cctx.enter_context(tc.tile_pool(name="o_psum", bufs=2, space="PSUM"))

    def head_ctx(br, h):
        kb_h, base = h // 2, (h % 2) * 64
        return (KT[br][base:base + 64, kb_h, :], QT[br][base:base + 64, kb_h, :],
                kb_h, base)

    for br in ("vis", "ir"):
        for j in range(H // 2):
            pair = (2 * j, 2 * j + 1)
            # --- per-q mean broadcasts (prefetchable) ---
            bmu = {}
            for h in pair:
                mu_row = row_pool.tile([1, N], F16, tag="mu_row")
                nc.sync.dma_start(mu_row[:], mu16[br][h:h + 1, :])
                bmu[h] = bmu_pool.tile([128, N], F16, tag="bmu", name="bmu")
                nc.gpsimd.partition_broadcast(bmu[h][:], mu_row[:])

            # --- scores, pair-interleaved for PE row-group overlap ---
            e16 = {h: c2_pool.tile([128, NT, N], F16, tag="e16", name="e16")
                   for h in pair}
            for kt in range(NT):
                for c in range(CH):
                    eps = {}
                    for h in pair:
                        kt_h, qt_h, _, _ = head_ctx(br, h)
                        eps[h] = eps_pool.tile([128, 512], F32, tag="e_ps",
                                               name="e_ps")
                        nc.tensor.matmul(
                            eps[h][:],
                            kt_h[:, kt * 128:(kt + 1) * 128],
                            qt_h[:, c * 512:(c + 1) * 512],
                            start=True,
                            stop=True,
                        )
                    for h in pair:
                        dst = e16[h][:, kt, c * 512:(c + 1) * 512]
                        if (kt + c) % 2 == 0:
                            nc.vector.tensor_copy(dst, eps[h][:])
                        else:
                            nc.scalar.copy(dst, eps[h][:])

            # --- in-place chain: tc = e-mu; tc*=tc; var; tc*=r; sigmoid;
            #     tc = e*tc; ew = exp(tc) ---
            tcx = {h: c_pool.tile([128, NT, N], F16, tag="tc", name="tc")
                   for h in pair}
            for h in pair:
                for kt in range(NT):
                    nc.vector.tensor_sub(
                        tcx[h][:, kt, :], e16[h][:, kt, :], bmu[h][:]
                    )
            for h in pair:
                nc.vector.tensor_mul(tcx[h][:], tcx[h][:], tcx[h][:])
            brx = {}
            for h in pair:
                vps = {}
                for c in range(CH):
                    vps[c] = aux_pool.tile([1, 512], F32, tag="red_ps",
                                           name="red_ps")
                    for kt in range(NT):
                        nc.tensor.matmul(
                            vps[c][:],
                            ones_f16[:],
                            tcx[h][:, kt, c * 512:(c + 1) * 512],
                            start=(kt == 0),
                            stop=(kt == NT - 1),
                        )
                rr = row_pool.tile([1, N], F32, tag="rr")
                for c in range(CH):
                    nc.vector.tensor_scalar(
                        rr[:, c * 512:(c + 1) * 512], vps[c][:],
                        2.0 / N, 1e-6, OP.mult, OP.add,
                    )
                rf = row_pool.tile([1, N], F32, tag="rf")
                nc.vector.reciprocal(rf[:], rr[:])
                r16row = row_pool.tile([1, N], F16, tag="r16row")
                nc.vector.tensor_copy(r16row[:], rf[:])
                brx[h] = c_pool.tile([128, N], F16, tag="br16", name="br16")
                nc.gpsimd.partition_broadcast(brx[h][:], r16row[:])
            for h in pair:
                for kt in range(NT):
                    nc.vector.tensor_mul(
                        tcx[h][:, kt, :], tcx[h][:, kt, :], brx[h][:]
                    )
            for h in pair:
                nc.scalar.activation(tcx[h][:], tcx[h][:], AF.Sigmoid,
                                     bias=c_half[:])
            for h in pair:
                nc.vector.tensor_mul(tcx[h][:], e16[h][:], tcx[h][:])
            ew = {}
            for h in pair:
                ew[h] = c2_pool.tile([128, NT, N], BF16, tag="e16", name="ew")
                nc.scalar.activation(ew[h][:], tcx[h][:], AF.Exp)

            # --- softmax denominator (ones-reduce) ---
            brd = {}
            for h in pair:
                dps = {}
                for c in range(CH):
                    dps[c] = aux_pool.tile([1, 512], F32, tag="red_ps",
                                           name="red_ps")
                    for kt in range(NT):
                        nc.tensor.matmul(
                            dps[c][:],
                            ones_bf[:],
                            ew[h][:, kt, c * 512:(c + 1) * 512],
                            start=(kt == 0),
                            stop=(kt == NT - 1),
                        )
                dd = row_pool.tile([1, N], F32, tag="rr")
                for c in range(CH):
                    nc.vector.tensor_copy(dd[:, c * 512:(c + 1) * 512], dps[c][:])
                rd = row_pool.tile([1, N], F32, tag="rf")
                nc.vector.reciprocal(rd[:], dd[:])
                brd[h] = c_pool.tile([128, N], F32, tag="brd", name="brd")
                nc.gpsimd.partition_broadcast(brd[h][:], rd[:])

            # --- AV, pair-packed into psum column groups ---
            for c in range(CH):
                ops = o_pool.tile([128, 512], F32, tag="o_ps", name="o_ps")
                for kt in range(NT):
                    for h in pair:
                        base_o = (h % 2) * 64
                        nc.tensor.matmul(
                            ops[base_o:base_o + 64, :],
                            Vn[br][:, kt, h * 64:(h + 1) * 64],
                            ew[h][:, kt, c * 512:(c + 1) * 512],
                            start=(kt == 0),
                            stop=(kt == NT - 1),
                            tile_position=(0, base_o),
                            skip_group_check=True,
                        )
                for h in pair:
                    _, _, kb_h, base = head_ctx(br, h)
                    base_o = (h % 2) * 64
                    nc.vector.scalar_tensor_tensor(
                        OT[br][base:base + 64, kb_h, c * 512:(c + 1) * 512],
                        ops[base_o:base_o + 64, :],
                        1.0,
                        brd[h][:64, c * 512:(c + 1) * 512],
                        OP.mult,
                        OP.mult,
                    )

    cctx.close()

    # =================== Stage D: out-proj (transposed output) ============
    with ExitStack() as sctx:
        w_pool = sctx.enter_context(tc.tile_pool(name="wts_out", bufs=2))
        d_pool = sctx.enter_context(tc.tile_pool(name="stD", bufs=4))
        dp_pool = sctx.enter_context(
            tc.tile_pool(name="stD_psum", bufs=4, space="PSUM")
        )
        for br in ("vis", "ir"):
            wname = "out_" + br
            w32 = w_pool.tile([128, DB, D], F32, tag="Wout32")
            nc.sync.dma_start(
                w32[:], io["W_" + wname][:].rearrange("(a p) o -> p a o", p=128)
            )
            w = w_pool.tile([128, DB, D], F16, tag="Wout")
            nc.vector.tensor_copy(w[:], w32[:])
            bout = cols["b_" + wname]
            bv = cols["b_v_" + br]
            # total bias = b_out + W_out^T b_v   (V-projection bias folded in)
            btot = d_pool.tile([128, DB], F32, tag="btot")
            for m in range(DB):
                wb = dp_pool.tile([128, 1], F32, tag="wb_ps")
                for kb in range(DB):
                    nc.tensor.matmul(
                        wb[:],
                        w32[:, kb, m * 128:(m + 1) * 128],
                        bv[:, kb:kb + 1],
                        start=(kb == 0),
                        stop=(kb == DB - 1),
                    )
                nc.vector.tensor_add(btot[:, m:m + 1], wb[:], bout[:, m:m + 1])
            out_dram = io["out_vis_T"] if br == "vis" else io["out_ir_T"]
            for m in range(DB):
                for c in range(CH):
                    ps = dp_pool.tile([128, 512], F32, tag="op_ps")
                    for kb in range(DB):
                        nc.tensor.matmul(
                            ps[:],
                            w[:, kb, m * 128:(m + 1) * 128],
                            OT[br][:, kb, c * 512:(c + 1) * 512],
                            start=(kb == 0),
                            stop=(kb == DB - 1),
                        )
                    osb = d_pool.tile([128, 512], F32, tag="osb")
                    nc.vector.tensor_scalar_add(osb[:], ps[:], btot[:, m:m + 1])
                    nc.sync.dma_start(
                        out_dram[m * 128:(m + 1) * 128, c * 512:(c + 1) * 512],
                        osb[:],
                    )


def build_nc():
    nc = bacc.Bacc()
    io = {}
    for nm in ["rgb_fea", "ir_fea", "rgb_fused", "ir_fused"]:
        io[nm] = nc.declare_dram_parameter(nm, [N, D], F32, isOutput=False)
    for nm in W_NAMES:
        io["W_" + nm] = nc.declare_dram_parameter("W_" + nm, [D, D], F32, isOutput=False)
        io["b_" + nm] = nc.declare_dram_parameter("b_" + nm, [D], F32, isOutput=False)
    for nm in ["ln1_g", "ln1_b", "ln2_g", "ln2_b"]:
        io[nm] = nc.declare_dram_parameter(nm, [D], F32, isOutput=False)
    io["out_vis_T"] = nc.declare_dram_parameter("out_vis_T", [D, N], F32, isOutput=True)
    io["out_ir_T"] = nc.declare_dram_parameter("out_ir_T", [D, N], F32, isOutput=True)

    with tile.TileContext(nc) as tc:
        with ExitStack() as ctx:
            _emit(ctx, tc, io)
    nc.finalize()
    return nc


_NC_CACHE = None


def _get_nc():
    global _NC_CACHE
    if _NC_CACHE is None:
        _NC_CACHE = build_nc()
    return _NC_CACHE


def _in_maps(rgb_fea, ir_fea, rgb_fused, ir_fused, params):
    maps = []
    for i in range(B):
        m = {
            "rgb_fea": np.ascontiguousarray(rgb_fea[i], np.float32),
            "ir_fea": np.ascontiguousarray(ir_fea[i], np.float32),
            "rgb_fused": np.ascontiguousarray(rgb_fused[i], np.float32),
            "ir_fused": np.ascontiguousarray(ir_fused[i], np.float32),
        }
        for nm in W_NAMES:
            m["W_" + nm] = np.ascontiguousarray(params["W_" + nm], np.float32)
            m["b_" + nm] = np.ascontiguousarray(params["b_" + nm], np.float32)
        for nm in ["ln1_g", "ln1_b", "ln2_g", "ln2_b"]:
            m[nm] = np.ascontiguousarray(params[nm], np.float32)
        maps.append(m)
    return maps


def run(rgb_fea, ir_fea, rgb_fused, ir_fused, params, trace=False):
    nc = _get_nc()
    maps = _in_maps(
        np.asarray(rgb_fea), np.asarray(ir_fea),
        np.asarray(rgb_fused), np.asarray(ir_fused), params,
    )
    res = run_bass_kernel_spmd(nc, maps, list(range(B)), trace=trace)
    out_vis = np.stack([res.results[i]["out_vis_T"].T for i in range(B)])
    out_ir = np.stack([res.results[i]["out_ir_T"].T for i in range(B)])
    return (out_vis, out_ir), res


def kernel(rgb_fea, ir_fea, rgb_fused, ir_fused, params):
    (out_vis, out_ir), _ = run(rgb_fea, ir_fea, rgb_fused, ir_fused, params)
    return out_vis, out_ir


# revision 11
# speedup vs baseline: 1.4998x; 1.1716x over previous
"""CrossVarianceAttention Trainium2 kernel.

Sharding: data-parallel over batch B=8, one batch element per NeuronCore
(8 cores). Each core computes the full two-branch cross-attention for its
batch element; outputs are gathered (and transposed) on host.

Device layout notes (per core, one batch element):
  - activations [1024, 512] are transposed on-device to [512, 1024]
    ("T layout": feature on partitions) because every matmul contracts
    over features.
  - attention runs per (branch, head) in [k, q] layout (k on partitions)
    so that att @ V needs no transpose; per-q statistics (mean/var of the
    variance-weighting and the softmax denominator) are computed with
    tensor-engine ones/B-matrix reductions, then broadcast across
    partitions with gpsimd partition_broadcast.
  - final out_proj produces out^T [512, 1024]; host transposes back.
"""

import os
import sys
from contextlib import ExitStack

import numpy as np

for _p in ("/opt/trn_rl_repo", "/root/.axon_site/_ro/trn_rl_repo"):
    if os.path.isdir(_p) and _p not in sys.path:
        sys.path.insert(0, _p)

import concourse.bass as bass
import concourse.bacc as bacc
import concourse.mybir as mybir
from concourse import tile
from concourse.bass_utils import run_bass_kernel_spmd
from concourse.masks import make_identity

F32 = mybir.dt.float32
F16 = mybir.dt.float16
BF16 = mybir.dt.bfloat16
AX = mybir.AxisListType
OP = mybir.AluOpType
AF = mybir.ActivationFunctionType

B, N, D = 8, 1024, 512
H, DK = 8, 64
NT = N // 128          # 8 n/k tiles of 128
DB = D // 128          # 4 feature blocks of 128
CH = N // 512          # 2 free-dim chunks of 512 (fp32 matmul N limit)
SCALE = float(np.sqrt(DK))
LN_EPS = 1e-5

W_NAMES = ["q_vis", "k_vis", "v_vis", "q_ir", "k_ir", "v_ir", "out_vis", "out_ir"]


def _emit(ctx: ExitStack, tc: "tile.TileContext", io: dict):
    nc = tc.nc

    const_pool = ctx.enter_context(tc.tile_pool(name="const", bufs=1))
    ident = const_pool.tile([128, 128], F32)
    make_identity(nc, ident[:])
    ones_f16 = const_pool.tile([128, 1], F16)
    nc.vector.memset(ones_f16[:], 1.0)
    ones_bf = const_pool.tile([128, 1], BF16)
    nc.vector.memset(ones_bf[:], 1.0)
    c_eps = const_pool.tile([128, 1], F32)
    nc.vector.memset(c_eps[:], LN_EPS)
    c_half = const_pool.tile([128, 1], F32)
    nc.vector.memset(c_half[:], 0.5)

    # --- load per-feature vectors as [128, DB] columns ---
    def load_cols(name):
        t = const_pool.tile([128, DB], F32, tag=f"col_{name}", name=f"col_{name}")
        nc.sync.dma_start(t[:], io[name][:].rearrange("(a p) -> p a", p=128))
        return t

    cols = {}
    for nm in ["ln1_g", "ln1_b", "ln2_g", "ln2_b"]:
        cols[nm] = load_cols(nm)
    for nm in W_NAMES:
        cols["b_" + nm] = load_cols("b_" + nm)

    # persistent projection outputs
    projT_pool = ctx.enter_context(tc.tile_pool(name="projT", bufs=1))
    QT = {}   # [128, DB, N] f16 : Q^T/SCALE per branch (branch -> tile)
    KT = {}   # [128, DB, N] f16 : K^T per branch
    Vn = {}   # [128, NT, D] bf16: V natural per branch
    for br in ("vis", "ir"):
        QT[br] = projT_pool.tile([128, DB, N], F16, tag=f"QT_{br}", name=f"QT_{br}")
        KT[br] = projT_pool.tile([128, DB, N], F16, tag=f"KT_{br}", name=f"KT_{br}")
        Vn[br] = projT_pool.tile([128, NT, D], BF16, tag=f"V_{br}", name=f"V_{br}")

    ot_pool = ctx.enter_context(tc.tile_pool(name="ot", bufs=1))

    stats_pool = ctx.enter_context(tc.tile_pool(name="stats", bufs=1))
    mu16 = {br: stats_pool.tile([H, N], F16, tag=f"mu16_{br}", name=f"mu16_{br}") for br in ("vis", "ir")}

    # =================== Stage A: LN + transpose inputs ===================
    with ExitStack() as sctx:
        inT_pool = sctx.enter_context(tc.tile_pool(name="inT", bufs=1))
        xT = {}
        for nm in ("lnT_vis", "lnT_ir", "fusT_rgb", "fusT_ir"):
            xT[nm] = inT_pool.tile([128, DB, N], F32, tag=nm, name=nm)

        a_pool = sctx.enter_context(tc.tile_pool(name="stA", bufs=4))
        st_pool = sctx.enter_context(tc.tile_pool(name="stA_stats", bufs=8))
        pT_pool = sctx.enter_context(
            tc.tile_pool(name="stA_psum", bufs=2, space="PSUM")
        )

        def ln_transpose(src_ap, gname, bname, dst):
            g, b = cols[gname], cols[bname]
            for nt in range(NT):
                x = a_pool.tile([128, D], F32, tag="x_in")
                nc.sync.dma_start(x[:], src_ap[nt * 128:(nt + 1) * 128, :])
                ssum = st_pool.tile([128, 1], F32, tag="ssum")
                nc.vector.tensor_reduce(ssum[:], x[:], AX.X, OP.add)
                sq = a_pool.tile([128, D], F32, tag="sq_scratch")
                sqsum = st_pool.tile([128, 1], F32, tag="sqsum")
                nc.scalar.activation(sq[:], x[:], AF.Square, accum_out=sqsum[:])
                mu = st_pool.tile([128, 1], F32, tag="mu")
                nc.vector.tensor_scalar_mul(mu[:], ssum[:], 1.0 / D)
                ex2 = st_pool.tile([128, 1], F32, tag="ex2")
                nc.vector.tensor_scalar_mul(ex2[:], sqsum[:], 1.0 / D)
                mu2 = st_pool.tile([128, 1], F32, tag="mu2")
                nc.vector.tensor_mul(mu2[:], mu[:], mu[:])
                var = st_pool.tile([128, 1], F32, tag="var")
                nc.vector.tensor_sub(var[:], ex2[:], mu2[:])
                std = st_pool.tile([128, 1], F32, tag="std")
                nc.scalar.activation(std[:], var[:], AF.Sqrt, bias=c_eps[:])
                rstd = st_pool.tile([128, 1], F32, tag="rstd")
                nc.vector.reciprocal(rstd[:], std[:])
                xh = a_pool.tile([128, D], F32, tag="xhat")
                nc.vector.tensor_scalar(
                    xh[:], x[:], mu[:], rstd[:], OP.subtract, OP.mult
                )
                for kb in range(DB):
                    ps = pT_pool.tile([128, 128], F32, tag="pT")
                    nc.tensor.transpose(
                        ps[:], xh[:, kb * 128:(kb + 1) * 128], ident[:]
                    )
                    nc.vector.tensor_scalar(
                        dst[:, kb, nt * 128:(nt + 1) * 128],
                        ps[:],
                        g[:, kb:kb + 1],
                        b[:, kb:kb + 1],
                        OP.mult,
                        OP.add,
                    )

        def plain_transpose(src_ap, dst):
            for nt in range(NT):
                x = a_pool.tile([128, D], F32, tag="x_in")
                nc.sync.dma_start(x[:], src_ap[nt * 128:(nt + 1) * 128, :])
                for kb in range(DB):
                    ps = pT_pool.tile([128, 128], F32, tag="pT")
                    nc.tensor.transpose(
                        ps[:], x[:, kb * 128:(kb + 1) * 128], ident[:]
                    )
                    nc.scalar.copy(dst[:, kb, nt * 128:(nt + 1) * 128], ps[:])

        ln_transpose(io["rgb_fea"][:], "ln1_g", "ln1_b", xT["lnT_vis"])
        ln_transpose(io["ir_fea"][:], "ln2_g", "ln2_b", xT["lnT_ir"])
        plain_transpose(io["rgb_fused"][:], xT["fusT_rgb"])
        plain_transpose(io["ir_fused"][:], xT["fusT_ir"])

        # =================== Stage B: projections ===================
        w_pool = sctx.enter_context(tc.tile_pool(name="wts", bufs=2))
        pj_pool = sctx.enter_context(
            tc.tile_pool(name="stB_psum", bufs=4, space="PSUM")
        )

        def load_w(name):
            w = w_pool.tile([128, DB, D], F32, tag="W")
            nc.sync.dma_start(
                w[:], io["W_" + name][:].rearrange("(a p) o -> p a o", p=128)
            )
            return w

        def proj_T(xt, wname, dst, scale=None):
            w = load_w(wname)
            bc = cols["b_" + wname]
            for m in range(DB):
                for c in range(CH):
                    ps = pj_pool.tile([128, 512], F32, tag="pj")
                    for kb in range(DB):
                        nc.tensor.matmul(
                            ps[:],
                            w[:, kb, m * 128:(m + 1) * 128],
                            xt[:, kb, c * 512:(c + 1) * 512],
                            start=(kb == 0),
                            stop=(kb == DB - 1),
                        )
                    if scale is None:
                        nc.vector.tensor_scalar_add(
                            dst[:, m, c * 512:(c + 1) * 512], ps[:], bc[:, m:m + 1]
                        )
                    else:
                        nc.vector.tensor_scalar(
                            dst[:, m, c * 512:(c + 1) * 512],
                            ps[:],
                            bc[:, m:m + 1],
                            float(scale),
                            OP.add,
                            OP.mult,
                        )

        def proj_N(xt, wname, dst):
            # natural-layout projection (for V), bias deferred to host-side
            # identity:   sum_k P[k,q] = 1  =>  bias handled via +b after
            # normalization (added on device in out-proj stage via W^T b).
            w = load_w(wname)
            for nt in range(NT):
                ps = pj_pool.tile([128, 512], F32, tag="pj")
                for kb in range(DB):
                    nc.tensor.matmul(
                        ps[:],
                        xt[:, kb, nt * 128:(nt + 1) * 128],
                        w[:, kb, :],
                        start=(kb == 0),
                        stop=(kb == DB - 1),
                    )
                nc.vector.tensor_copy(dst[:, nt, :], ps[:])

        # branch "vis": Q from ir_fused (W_q_ir), K/V from LN(rgb_fea)
        proj_T(xT["fusT_ir"], "q_ir", QT["vis"], scale=1.0 / SCALE)
        proj_T(xT["lnT_vis"], "k_vis", KT["vis"])
        proj_N(xT["lnT_vis"], "v_vis", Vn["vis"])
        # branch "ir": Q from rgb_fused (W_q_vis), K/V from LN(ir_fea)
        proj_T(xT["fusT_rgb"], "q_vis", QT["ir"], scale=1.0 / SCALE)
        proj_T(xT["lnT_ir"], "k_ir", KT["ir"])
        proj_N(xT["lnT_ir"], "v_ir", Vn["ir"])

        # ---- Stage B2: column means of scores via B-matrix trick ----
        # mu[h, q] = (sum_k e[k, q]) / N = (ksum_head . Q^T_head)[q] / N
        b2_pool = sctx.enter_context(tc.tile_pool(name="stB2", bufs=1))
        mu_ps_pool = sctx.enter_context(
            tc.tile_pool(name="stB2_psum", bufs=1, space="PSUM")
        )
        for br in ("vis", "ir"):
            ks = b2_pool.tile([128, DB], F32, tag="ksum")
            for kb in range(DB):
                nc.vector.tensor_reduce(
                    ks[:, kb:kb + 1], KT[br][:, kb, :], AX.X, OP.add
                )
            bmat = b2_pool.tile([128, DB, H], F16, tag="bmat")
            nc.vector.memset(bmat[:], 0.0)
            for h in range(H):
                kb_h, base = h // 2, (h % 2) * 64
                nc.vector.tensor_copy(
                    bmat[base:base + 64, kb_h, h:h + 1],
                    ks[base:base + 64, kb_h:kb_h + 1],
                )
            mps = mu_ps_pool.tile([H, N], F32, tag="mu_ps")
            for c in range(CH):
                for kb in range(DB):
                    nc.tensor.matmul(
                        mps[:, c * 512:(c + 1) * 512],
                        bmat[:, kb, :],
                        QT[br][:, kb, c * 512:(c + 1) * 512],
                        start=(kb == 0),
                        stop=(kb == DB - 1),
                    )
            nc.vector.tensor_scalar_mul(mu16[br][:], mps[:], 1.0 / N)

    # =================== Stage C: attention per (branch, head) ============
    OT = {br: ot_pool.tile([128, DB, N], F16, tag=f"OT_{br}", name=f"OT_{br}") for br in ("vis", "ir")}
    cctx = ctx.enter_context(ExitStack())
    c_pool = cctx.enter_context(tc.tile_pool(name="stC", bufs=2))
    c2_pool = cctx.enter_context(tc.tile_pool(name="stC_e", bufs=3))
    bmu_pool = cctx.enter_context(tc.tile_pool(name="stC_bmu", bufs=4))
    row_pool = cctx.enter_context(tc.tile_pool(name="stC_rows", bufs=2))
    eps_pool = cctx.enter_context(tc.tile_pool(name="e_psum", bufs=4, space="PSUM"))
    aux_pool = cctx.enter_context(tc.tile_pool(name="aux_psum", bufs=2, space="PSUM"))
    o_pool = cctx.enter_context(tc.tile_pool(name="o_psum", bufs=2, space="PSUM"))

    def head_ctx(br, h):
        kb_h, base = h // 2, (h % 2) * 64
        return (KT[br][base:base + 64, kb_h, :], QT[br][base:base + 64, kb_h, :],
                kb_h, base)

    for br in ("vis", "ir"):
        for j in range(H // 2):
            pair = (2 * j, 2 * j + 1)
            # --- per-q mean broadcasts (prefetchable) ---
            bmu = {}
            for h in pair:
                mu_row = row_pool.tile([1, N], F16, tag="mu_row")
                nc.sync.dma_start(mu_row[:], mu16[br][h:h + 1, :])
                bmu[h] = bmu_pool.tile([128, N], F16, tag="bmu", name="bmu")
                nc.gpsimd.partition_broadcast(bmu[h][:], mu_row[:])

            # --- scores, pair-interleaved for PE row-group overlap ---
            e16 = {h: c2_pool.tile([128, NT, N], F16, tag="e16", name="e16")
                   for h in pair}
            for kt in range(NT):
                for c in range(CH):
                    eps = {}
                    for h in pair:
                        kt_h, qt_h, _, _ = head_ctx(br, h)
                        eps[h] = eps_pool.tile([128, 512], F32, tag="e_ps",
                                               name="e_ps")
                        nc.tensor.matmul(
                            eps[h][:],
                            kt_h[:, kt * 128:(kt + 1) * 128],
                            qt_h[:, c * 512:(c + 1) * 512],
                            start=True,
                            stop=True,
                        )
                    for h in pair:
                        dst = e16[h][:, kt, c * 512:(c + 1) * 512]
                        if (kt + c) % 2 == 0:
                            nc.vector.tensor_copy(dst, eps[h][:])
                        else:
                            nc.scalar.copy(dst, eps[h][:])

            # --- in-place chain: tc = e-mu; tc*=tc; var; tc*=r; sigmoid;
            #     tc = e*tc; ew = exp(tc) ---
            tcx = {h: c_pool.tile([128, NT, N], F16, tag="tc", name="tc")
                   for h in pair}
            for h in pair:
                bmu_b = bmu[h][:].rearrange("p (o n) -> p o n", o=1).to_broadcast(
                    [128, NT, N])
                nc.vector.tensor_sub(tcx[h][:], e16[h][:], bmu_b)
            for h in pair:
                nc.vector.tensor_mul(tcx[h][:], tcx[h][:], tcx[h][:])
            brx = {}
            for h in pair:
                vps = {}
                for c in range(CH):
                    vps[c] = aux_pool.tile([1, 512], F32, tag="red_ps",
                                           name="red_ps")
                    for kt in range(NT):
                        nc.tensor.matmul(
                            vps[c][:],
                            ones_f16[:],
                            tcx[h][:, kt, c * 512:(c + 1) * 512],
                            start=(kt == 0),
                            stop=(kt == NT - 1),
                        )
                rr = row_pool.tile([1, N], F32, tag="rr")
                for c in range(CH):
                    nc.vector.tensor_scalar(
                        rr[:, c * 512:(c + 1) * 512], vps[c][:],
                        2.0 / N, 1e-6, OP.mult, OP.add,
                    )
                rf = row_pool.tile([1, N], F32, tag="rf")
                nc.vector.reciprocal_approx_fast(rf[:], rr[:])
                r16row = row_pool.tile([1, N], F16, tag="r16row")
                nc.vector.tensor_copy(r16row[:], rf[:])
                brx[h] = c_pool.tile([128, N], F16, tag="br16", name="br16")
                nc.gpsimd.partition_broadcast(brx[h][:], r16row[:])
            for h in pair:
                br_b = brx[h][:].rearrange("p (o n) -> p o n", o=1).to_broadcast(
                    [128, NT, N])
                nc.vector.tensor_mul(tcx[h][:], tcx[h][:], br_b)
            for h in pair:
                nc.scalar.activation(tcx[h][:], tcx[h][:], AF.Sigmoid,
                                     bias=c_half[:])
            for h in pair:
                nc.vector.tensor_mul(tcx[h][:], e16[h][:], tcx[h][:])
            ew = {}
            for h in pair:
                ew[h] = c2_pool.tile([128, NT, N], BF16, tag="e16", name="ew")
                nc.scalar.activation(ew[h][:], tcx[h][:], AF.Exp)

            # --- softmax denominator (ones-reduce) ---
            brd = {}
            for h in pair:
                dps = {}
                for c in range(CH):
                    dps[c] = aux_pool.tile([1, 512], F32, tag="red_ps",
                                           name="red_ps")
                    for kt in range(NT):
                        nc.tensor.matmul(
                            dps[c][:],
                            ones_bf[:],
                            ew[h][:, kt, c * 512:(c + 1) * 512],
                            start=(kt == 0),
                            stop=(kt == NT - 1),
                        )
                dd = row_pool.tile([1, N], F32, tag="rr")
                for c in range(CH):
                    nc.vector.tensor_copy(dd[:, c * 512:(c + 1) * 512], dps[c][:])
                rd = row_pool.tile([1, N], F32, tag="rf")
                nc.vector.reciprocal_approx_fast(rd[:], dd[:])
                brd[h] = c_pool.tile([128, N], F32, tag="brd", name="brd")
                nc.gpsimd.partition_broadcast(brd[h][:], rd[:])

            # --- AV, pair-packed into psum column groups ---
            for c in range(CH):
                ops = o_pool.tile([128, 512], F32, tag="o_ps", name="o_ps")
                for kt in range(NT):
                    for h in pair:
                        base_o = (h % 2) * 64
                        nc.tensor.matmul(
                            ops[base_o:base_o + 64, :],
                            Vn[br][:, kt, h * 64:(h + 1) * 64],
                            ew[h][:, kt, c * 512:(c + 1) * 512],
                            start=(kt == 0),
                            stop=(kt == NT - 1),
                            tile_position=(0, base_o),
                            skip_group_check=True,
                        )
                for h in pair:
                    _, _, kb_h, base = head_ctx(br, h)
                    base_o = (h % 2) * 64
                    nc.vector.scalar_tensor_tensor(
                        OT[br][base:base + 64, kb_h, c * 512:(c + 1) * 512],
                        ops[base_o:base_o + 64, :],
                        1.0,
                        brd[h][:64, c * 512:(c + 1) * 512],
                        OP.mult,
                        OP.mult,
                    )

    cctx.close()

    # =================== Stage D: out-proj (transposed output) ============
    with ExitStack() as sctx:
        w_pool = sctx.enter_context(tc.tile_pool(name="wts_out", bufs=2))
        d_pool = sctx.enter_context(tc.tile_pool(name="stD", bufs=4))
        dp_pool = sctx.enter_context(
            tc.tile_pool(name="stD_psum", bufs=4, space="PSUM")
        )
        for br in ("vis", "ir"):
            wname = "out_" + br
            w32 = w_pool.tile([128, DB, D], F32, tag="Wout32")
            nc.sync.dma_start(
                w32[:], io["W_" + wname][:].rearrange("(a p) o -> p a o", p=128)
            )
            w = w_pool.tile([128, DB, D], F16, tag="Wout")
            nc.vector.tensor_copy(w[:], w32[:])
            bout = cols["b_" + wname]
            bv = cols["b_v_" + br]
            # total bias = b_out + W_out^T b_v   (V-projection bias folded in)
            btot = d_pool.tile([128, DB], F32, tag="btot")
            for m in range(DB):
                wb = dp_pool.tile([128, 1], F32, tag="wb_ps")
                for kb in range(DB):
                    nc.tensor.matmul(
                        wb[:],
                        w32[:, kb, m * 128:(m + 1) * 128],
                        bv[:, kb:kb + 1],
                        start=(kb == 0),
                        stop=(kb == DB - 1),
                    )
                nc.vector.tensor_add(btot[:, m:m + 1], wb[:], bout[:, m:m + 1])
            out_dram = io["out_vis_T"] if br == "vis" else io["out_ir_T"]
            for m in range(DB):
                for c in range(CH):
                    ps = dp_pool.tile([128, 512], F32, tag="op_ps")
                    for kb in range(DB):
                        nc.tensor.matmul(
                            ps[:],
                            w[:, kb, m * 128:(m + 1) * 128],
                            OT[br][:, kb, c * 512:(c + 1) * 512],
                            start=(kb == 0),
                            stop=(kb == DB - 1),
                        )
                    osb = d_pool.tile([128, 512], F32, tag="osb")
                    nc.vector.tensor_scalar_add(osb[:], ps[:], btot[:, m:m + 1])
                    nc.sync.dma_start(
                        out_dram[m * 128:(m + 1) * 128, c * 512:(c + 1) * 512],
                        osb[:],
                    )


def build_nc():
    nc = bacc.Bacc()
    io = {}
    for nm in ["rgb_fea", "ir_fea", "rgb_fused", "ir_fused"]:
        io[nm] = nc.declare_dram_parameter(nm, [N, D], F32, isOutput=False)
    for nm in W_NAMES:
        io["W_" + nm] = nc.declare_dram_parameter("W_" + nm, [D, D], F32, isOutput=False)
        io["b_" + nm] = nc.declare_dram_parameter("b_" + nm, [D], F32, isOutput=False)
    for nm in ["ln1_g", "ln1_b", "ln2_g", "ln2_b"]:
        io[nm] = nc.declare_dram_parameter(nm, [D], F32, isOutput=False)
    io["out_vis_T"] = nc.declare_dram_parameter("out_vis_T", [D, N], F32, isOutput=True)
    io["out_ir_T"] = nc.declare_dram_parameter("out_ir_T", [D, N], F32, isOutput=True)

    with tile.TileContext(nc) as tc:
        with ExitStack() as ctx:
            _emit(ctx, tc, io)
    nc.finalize()
    return nc


_NC_CACHE = None


def _get_nc():
    global _NC_CACHE
    if _NC_CACHE is None:
        _NC_CACHE = build_nc()
    return _NC_CACHE


def _in_maps(rgb_fea, ir_fea, rgb_fused, ir_fused, params):
    maps = []
    for i in range(B):
        m = {
            "rgb_fea": np.ascontiguousarray(rgb_fea[i], np.float32),
            "ir_fea": np.ascontiguousarray(ir_fea[i], np.float32),
            "rgb_fused": np.ascontiguousarray(rgb_fused[i], np.float32),
            "ir_fused": np.ascontiguousarray(ir_fused[i], np.float32),
        }
        for nm in W_NAMES:
            m["W_" + nm] = np.ascontiguousarray(params["W_" + nm], np.float32)
            m["b_" + nm] = np.ascontiguousarray(params["b_" + nm], np.float32)
        for nm in ["ln1_g", "ln1_b", "ln2_g", "ln2_b"]:
            m[nm] = np.ascontiguousarray(params[nm], np.float32)
        maps.append(m)
    return maps


def run(rgb_fea, ir_fea, rgb_fused, ir_fused, params, trace=False):
    nc = _get_nc()
    maps = _in_maps(
        np.asarray(rgb_fea), np.asarray(ir_fea),
        np.asarray(rgb_fused), np.asarray(ir_fused), params,
    )
    res = run_bass_kernel_spmd(nc, maps, list(range(B)), trace=trace)
    out_vis = np.stack([res.results[i]["out_vis_T"].T for i in range(B)])
    out_ir = np.stack([res.results[i]["out_ir_T"].T for i in range(B)])
    return (out_vis, out_ir), res


def kernel(rgb_fea, ir_fea, rgb_fused, ir_fused, params):
    (out_vis, out_ir), _ = run(rgb_fea, ir_fea, rgb_fused, ir_fused, params)
    return out_vis, out_ir
